# revision 33
# baseline (speedup 1.0000x reference)
"""Tensor-parallel attention forward (B=4, S=512, D=4096, H=32, HKV=8, HD=128,
START=512) on 8 TRN2 NeuronCores.

Sharding (chosen): TP over heads. Each core c owns q-heads 4c..4c+3 (wq rows
512c:512c+512), kv-head c (wk/wv rows 128c:128c+128, cache slice c), and
output columns 512c:512c+512 (wo rows 512c:512c+512). x is replicated. After
local attention, per-core attention outputs (head-sharded) are AllGathered
(bf16, split in two per token block for earlier comm start) and each core
computes its own 512-column slice of the output projection — no reduction
collective needed. The host concatenates the 8 column slices.

Host-side layout prep (part of sharding): operands are pre-transposed so the
contraction dim (model dim d / feature dim e) lands on SBUF partitions with
natural-stride DMA, pre-cast to bf16 (the on-device compute precision — this
halves HBM traffic), and RoPE pair dims of wq/wk/cache_k are pre-permuted to
[evens, odds] so the on-chip rotation is two contiguous 64-partition blocks.

Compute: bf16 matmuls (fp32 PSUM accumulate), fp32 softmax denominators.
Causal structure: key-tile kt >= 4 only attends to queries s >= 128*(kt-4);
matmul N, exp and denominator work are trimmed accordingly, and only the
128-wide diagonal block needs the affine predicate fill.
"""
import math

import numpy as np
import ml_dtypes

import concourse.mybir as mybir
from concourse import bass
from concourse.tile import TileContext
from concourse.bass_utils import run_bass_kernel_spmd

F32 = mybir.dt.float32
BF16 = mybir.dt.float16   # compute dtype (fp16: same PE speed, 8x less quant error)
F8 = mybir.dt.float8e3    # gather payload dtype: e3m4 (4 mantissa bits,
                          # range +-15.5 fits |at|<=8; ~1.3% rms)
PAY_DTYPE = F8            # set to BF16 to bisect fp8 payload issues
NPBF16 = np.float16
PAY_SCALE = 1.5           # 1.5*at: max ~12.1 < 15.5, trims e3m4 subnormal share
EXP_BIAS = -6.0           # exp(s/sqrt(hd) - 6): keeps fp16 pt in range
                          # (max score ~15 -> e^9 ~ 8.1e3 < 65504); the bias
                          # cancels exactly in pv/den

NCORES = 8
B, S, D = 4, 512, 4096
H, HKV, HD = 32, 8, 128
START = 512
T = START + S          # 1024 total kv length
NT = B * S             # 2048 tokens
NH = H // NCORES       # 4 local q heads
EL = NH * HD           # 512 local e width
DT = D // 128          # 32 d-tiles
KT = T // 128          # 8 k-tiles
NKC = START // 128     # 4 cached k-tiles
SCALE = 1.0 / math.sqrt(HD)

# RoPE pair permutation: head-dim reordered to [evens, odds]
PERM = np.concatenate([np.arange(0, HD, 2), np.arange(1, HD, 2)])

SPLIT_AG = True   # kept for compat; NGR is authoritative
NGR = 1           # gathers per token block (fp8 payload: 1 x 2MB slot)
PIPE_DEPTH = 1    # token blocks between a gather and its output projection
PT_BUFS = 2       # probability-tile double/triple buffering
TE_BUFS = 2       # exp-staging tiles for the masked diagonal
DUP_DVE = False   # diagnostic: double rope DVE work
DUP_ACT = False   # diagnostic: double exp work
DUP_POOL = False  # diagnostic: double affine_select work
DUP_COLL = False  # diagnostic: double collectives
COLL_PROBE = None  # diagnostic: None|"disj"|"quad"|"pair"|"a2a" collective probes
LOCAL_COLL = False  # diagnostic (sim): replace AllGather with local dram DMA
DBG_TAPS = False  # diagnostic: dump ag_in/ag_out of block 0
ITER_BARRIER = False  # diagnostic: all-engine barrier between iterations
                      # (steady-state delta then measures single-shot time)

_counter = [0]


def _dedup_ldweights(nc):
    """Drop InstLdweights whose stationary AP is identical to the previous
    PE weight load (weights persist in the PE array across matmuls)."""
    removed = 0
    for f in nc.m.functions:
        for blk in f.blocks:
            last_sig = None
            keep = []
            for inst in blk.instructions:
                tn = type(inst).__name__
                if tn == "InstLdweights":
                    sig = (str(inst.ins[0])
                           + str(getattr(inst, "tile_position", None))
                           + str(getattr(inst, "tile_size", None)))
                    if sig == last_sig and not (inst.sync_info and inst.sync_info.on_wait):
                        removed += 1
                        continue
                    last_sig = sig
                elif tn == "InstMatmult":
                    # f32 matmuls stay self-loading (no split LDW) and
                    # clobber the PE weight array; transpose-mode matmuls
                    # change array state too
                    try:
                        if getattr(inst, "is_transpose", False) or \
                                "float32" in str(inst.ins[1].dtype):
                            last_sig = None
                    except Exception:
                        last_sig = None
                elif getattr(inst, "engine", None) == mybir.EngineType.PE:
                    last_sig = None
                keep.append(inst)
            blk.instructions = keep
    return removed


def _split_excess_waits(nc, cap: int = 1):
    """This walrus build rejects instructions with >1 sync waits; split the
    extras into leading no-ops on the same engine."""
    for f in nc.m.functions:
        for blk in f.blocks:
            insts = blk.instructions
            i = 0
            while i < len(insts):
                inst = insts[i]
                si = inst.sync_info
                if si is not None and si.on_wait is not None and len(si.on_wait) > cap:
                    waits = list(si.on_wait)
                    extra, keep = waits[:-cap], waits[-cap:]
                    nops = []
                    for j in range(0, len(extra), cap):
                        _counter[0] += 1
                        nops.append(mybir.InstNoOp(
                            name=f"waitsplit-{_counter[0]}",
                            engine=inst.engine, ins=[], outs=[],
                            sync_info=mybir.SyncInfo(
                                on_wait=extra[j:j + cap], on_update=[]),
                        ))
                    inst.sync_info = mybir.SyncInfo(
                        on_wait=keep, on_update=list(si.on_update or []))
                    for k, nop in enumerate(nops):
                        insts.insert(i + k, nop)
                    i += len(nops)
                i += 1


def build_nc(iters: int = 1):
    nc = bass.Bass(num_devices=NCORES)

    xT = nc.declare_dram_parameter("xT", [D, NT], BF16, isOutput=False)
    wqT = nc.declare_dram_parameter("wqT", [D, EL], BF16, isOutput=False)
    wkT = nc.declare_dram_parameter("wkT", [D, HD], BF16, isOutput=False)
    wvT = nc.declare_dram_parameter("wvT", [D, HD], BF16, isOutput=False)
    woT = nc.declare_dram_parameter("woT", [D, EL], BF16, isOutput=False)
    ckT = nc.declare_dram_parameter("ckT", [B, HD, START], BF16, isOutput=False)
    cv = nc.declare_dram_parameter("cv", [B, START, HD], BF16, isOutput=False)
    cosT = nc.declare_dram_parameter("cosT", [HD // 2, S], BF16, isOutput=False)
    sinT = nc.declare_dram_parameter("sinT", [HD // 2, S], BF16, isOutput=False)
    out = nc.declare_dram_parameter("out", [EL, NT], F32, isOutput=True)
    if DBG_TAPS:
        dbg_agin = nc.declare_dram_parameter("dbg_agin", [NH * HD, S], PAY_DTYPE,
                                             isOutput=True)
        dbg_agout = nc.declare_dram_parameter("dbg_agout", [NCORES * NH * HD, S],
                                              PAY_DTYPE, isOutput=True)

    ngr = NGR
    hpg = NH // ngr  # heads per gather group
    ag_in = [[nc.dram_tensor(f"ag_in_{b}_{g}", [hpg * HD, S], PAY_DTYPE)
              for g in range(ngr)] for b in range(B)]
    ag_out = [[nc.dram_tensor(f"ag_out_{b}_{g}", [NCORES * hpg * HD, S], PAY_DTYPE,
                              addr_space="Shared") for g in range(ngr)]
              for b in range(B)]

    rg = [list(range(NCORES))]
    rg_quads = [[0, 1, 2, 3], [4, 5, 6, 7]]
    rg_pairs = [[0, 1], [2, 3], [4, 5], [6, 7]]
    if COLL_PROBE in ("disj", "quad"):
        dj_in = [[nc.dram_tensor(f"dj_in_{b}_{g}", [hpg * HD, S], BF16)
                  for g in range(ngr)] for b in range(B)]
        dj_out = [[nc.dram_tensor(f"dj_out_{b}_{g}", [4 * hpg * HD, S], BF16)
                   for g in range(ngr)] for b in range(B)]
    elif COLL_PROBE == "pair":
        dj_in = [[nc.dram_tensor(f"dj_in_{b}_{g}", [hpg * HD, S], BF16)
                  for g in range(ngr)] for b in range(B)]
        dj_out = [[nc.dram_tensor(f"dj_out_{b}_{g}", [2 * hpg * HD, S], BF16)
                   for g in range(ngr)] for b in range(B)]
    elif COLL_PROBE == "a2a":
        dj_in = [[nc.dram_tensor(f"dj_in_{b}_{g}", [8 * hpg * HD, S], BF16)
                  for g in range(ngr)] for b in range(B)]
        dj_out = [[nc.dram_tensor(f"dj_out_{b}_{g}", [8 * hpg * HD, S], BF16)
                   for g in range(ngr)] for b in range(B)]

    with TileContext(nc) as tc:
        with (
            tc.tile_pool(name="wpool", bufs=1) as wpool,
            tc.tile_pool(name="cpool", bufs=1) as cpool,
            tc.tile_pool(name="xpool", bufs=2) as xpool,
            tc.tile_pool(name="qkv", bufs=2) as qkv,
            tc.tile_pool(name="work", bufs=2) as work,
            tc.tile_pool(name="denp", bufs=1) as denp,
            tc.tile_pool(name="ptp", bufs=PT_BUFS) as ptp,
            tc.tile_pool(name="tep", bufs=TE_BUFS) as tep,
            tc.tile_pool(name="rope", bufs=2) as ropep,
            tc.tile_pool(name="gath", bufs=3) as gath,
            tc.tile_pool(name="ps", bufs=2, space="PSUM") as ps,
            tc.tile_pool(name="pspv", bufs=4, space="PSUM") as pspv,
        ):
            # ---- preamble (ordered by first consumption: K/V weights and
            # rope tables first, then wq by head-group column halves)
            wk_s = wpool.tile([128, DT, HD], BF16, tag="wk")
            nc.scalar.dma_start(out=wk_s[:, :, :],
                                in_=wkT[:, :].rearrange("(i p) e -> p i e", p=128))
            wv_s = wpool.tile([128, DT, HD], BF16, tag="wv")
            nc.scalar.dma_start(out=wv_s[:, :, :],
                                in_=wvT[:, :].rearrange("(i p) e -> p i e", p=128))
            cos_s = cpool.tile([64, S], BF16, tag="cos")
            nc.scalar.dma_start(out=cos_s[:, :], in_=cosT[:, :])
            sin_s = cpool.tile([64, S], BF16, tag="sin")
            nc.scalar.dma_start(out=sin_s[:, :], in_=sinT[:, :])
            wq_s = wpool.tile([128, DT, EL], BF16, tag="wq")
            for half in range(2):
                for q in range(2):
                    nc.scalar.dma_start(
                        out=wq_s[:, 16 * q:16 * q + 16,
                                 256 * half:256 * half + 256],
                        in_=wqT[:, 256 * half:256 * half + 256].rearrange(
                            "(i p) e -> p i e", p=128)[:, 16 * q:16 * q + 16, :])
            ones_m = cpool.tile([128, 128], F32, tag="onm")
            nc.vector.memset(ones_m[:, :], 1.0 / PAY_SCALE)
            bias_t = cpool.tile([128, 1], F32, tag="bias")
            nc.vector.memset(bias_t[:, :], EXP_BIAS)
            # wo is not needed until the first output projection — load late
            wo_s = wpool.tile([128, DT, EL], BF16, tag="wo")

            def load_wo():
                for q in range(4):
                    nc.scalar.dma_start(
                        out=wo_s[:, 8 * q:8 * q + 8, :],
                        in_=woT[:, :].rearrange("(i p) e -> p i e", p=128)[:, 8 * q:8 * q + 8, :])

            def rope(dst_a, dst_b, src):
                """dst = rotate(src); src [128, S] PSUM f32 with partitions
                [evens(a) 0:64, odds(b) 64:128]; dst bf16 [64, S] slices."""
                for _dup in range(2 if DUP_DVE else 1):
                    _rope1(dst_a, dst_b, src)

            def _rope1(dst_a, dst_b, src):
                a, bb = src[0:64, :], src[64:128, :]
                t1 = ropep.tile([64, S], F32, tag="rt1")
                t2 = ropep.tile([64, S], F32, tag="rt2")
                nc.vector.tensor_tensor(out=t1[:, :], in0=a, in1=cos_s[:, :],
                                        op=mybir.AluOpType.mult)
                nc.vector.tensor_tensor(out=t2[:, :], in0=bb, in1=sin_s[:, :],
                                        op=mybir.AluOpType.mult)
                nc.vector.tensor_tensor(out=dst_a, in0=t1[:, :], in1=t2[:, :],
                                        op=mybir.AluOpType.subtract)
                t3 = ropep.tile([64, S], F32, tag="rt3")
                t4 = ropep.tile([64, S], F32, tag="rt4")
                nc.vector.tensor_tensor(out=t3[:, :], in0=a, in1=sin_s[:, :],
                                        op=mybir.AluOpType.mult)
                nc.vector.tensor_tensor(out=t4[:, :], in0=bb, in1=cos_s[:, :],
                                        op=mybir.AluOpType.mult)
                nc.vector.tensor_tensor(out=dst_b, in0=t3[:, :], in1=t4[:, :],
                                        op=mybir.AluOpType.add)

            def emit_wo(b):
                """Output projection for block b from the gathers. Gather g's
                tile index i covers e-tile 4*(i//hpg) + hpg*g + i%hpg."""
                ps_y = [pspv.tile([128, S], F32, tag="pspv", name=f"psy{b}_{dj}")
                        for dj in range(4)]
                for g in range(ngr):
                    if COLL_PROBE == "quad":  # timing probe: consume quad AG
                        src = dj_out[b][g][:, :].rearrange("(i p) q -> p i q", p=128)
                    else:
                        src = ag_out[b][g][:, :].rearrange("(i p) q -> p i q", p=128)
                    for c in range(NCORES):  # one chunk per source core
                        # f8e3 moving feeds the PE directly (f16 stationary x
                        # f8 moving matmul is exact on HW, 1 cyc/row)
                        ag_t = gath.tile([128, hpg, S], PAY_DTYPE, tag="agt")
                        cc = c % 4 if COLL_PROBE == "quad" else c
                        nc.gpsimd.dma_start(out=ag_t[:, :, :],
                                            in_=src[:, hpg * cc:hpg * cc + hpg, :])
                        for dj in range(4):
                            for t2 in range(hpg):
                                i = 4 * c + hpg * g + t2
                                nc.tensor.matmul(
                                    ps_y[dj][:, :],
                                    wo_s[:, i, 128 * dj:128 * dj + 128],
                                    ag_t[:, t2, :],
                                    start=(g == 0 and c == 0 and t2 == 0),
                                    stop=(g == ngr - 1 and c == NCORES - 1
                                          and t2 == hpg - 1))
                for dj in range(4):
                    yt = work.tile([128, S], F32, tag="yt")
                    nc.scalar.activation(yt[:, :], ps_y[dj][:, :],
                                         mybir.ActivationFunctionType.Copy,
                                         scale=1.0 / PAY_SCALE)
                    nc.sync.dma_start(
                        out=out[128 * dj:128 * dj + 128, S * b:S * b + S],
                        in_=yt[:, :])

            pending = []
            for it in range(iters):
                if ITER_BARRIER and it > 0:
                    for pb in pending:
                        emit_wo(pb)
                    pending = []
                    nc.all_engine_barrier(sem_only=True)
                for b in range(B):
                    # ---- loads for this token block (= batch b) ----
                    xt0 = xpool.tile([128, DT // 2, S], BF16, tag="xt")
                    xt1 = xpool.tile([128, DT // 2, S], BF16, tag="xt")
                    xsrc = xT[:, S * b:S * b + S].rearrange("(i p) t -> p i t", p=128)
                    for hh in range(2):
                        nc.gpsimd.dma_start(out=xt0[:, 8 * hh:8 * hh + 8, :],
                                            in_=xsrc[:, 8 * hh:8 * hh + 8, :])
                    for hh in range(2):
                        nc.gpsimd.dma_start(out=xt1[:, 8 * hh:8 * hh + 8, :],
                                            in_=xsrc[:, 16 + 8 * hh:16 + 8 * hh + 8, :])

                    def xt(i):
                        return (xt0 if i < DT // 2 else xt1)[:, i % (DT // 2), :]

                    kT_b = qkv.tile([128, T], BF16, tag="kT")
                    nc.gpsimd.dma_start(out=kT_b[:, 0:START], in_=ckT[b])
                    v_b = qkv.tile([128, KT, HD], BF16, tag="v")
                    nc.gpsimd.dma_start(
                        out=v_b[:, 0:NKC, :],
                        in_=cv[b].rearrange("(kt p) dv -> p kt dv", p=128))
                    qT_b = qkv.tile([128, NH, S], BF16, tag="qT")

                    # ---- K and V projections, interleaved per d-tile (x is
                    # consumed at half the matmul rate: block-0 fill is
                    # DMA-paced, not stalled) ----
                    ps_k2 = ps.tile([128, 2, S], F32, tag="ps")
                    ps_k = ps_k2[:, 0, :]
                    ps_vt2 = ps.tile([128, 2, S], F32, tag="ps")
                    ps_vt = ps_vt2[:, 0, :]
                    for i in range(DT):
                        nc.tensor.matmul(ps_k, wk_s[:, i, :], xt(i),
                                         start=(i == 0), stop=(i == DT - 1))
                        nc.tensor.matmul(ps_vt, wv_s[:, i, :], xt(i),
                                         start=(i == 0), stop=(i == DT - 1))
                    rope(kT_b[0:64, START:T], kT_b[64:128, START:T], ps_k)
                    vT = work.tile([128, S], BF16, tag="vT")
                    nc.vector.tensor_copy(out=vT[:, :], in_=ps_vt)
                    for ts in range(S // 128):
                        nc.sync.dma_start(out=v_b[:, NKC + ts, :],
                                          in_=vT[:, 128 * ts:128 * ts + 128],
                                          transpose=True)

                    den = [denp.tile([128, 2, S], F32, tag=f"den{g}",
                                     name=f"den{b}_{g}") for g in range(2)]

                    def attn_group(g):
                        """Q proj + attention + normalize + gather for heads
                        2g, 2g+1 — the group's AllGather launches before the
                        other group's Q projection even starts."""
                        ps_qs = [pspv.tile([128, S], F32, tag="pspv",
                                           name=f"psq{b}_{g}_{j}")
                                 for j in range(2)]
                        for i in range(DT):
                            for j in range(2):
                                h = 2 * g + j
                                nc.tensor.matmul(
                                    ps_qs[j][:, :],
                                    wq_s[:, i, 128 * h:128 * h + 128],
                                    xt(i), start=(i == 0), stop=(i == DT - 1))
                        for j in range(2):
                            h = 2 * g + j
                            rope(qT_b[0:64, h, :], qT_b[64:128, h, :],
                                 ps_qs[j][:, :])

                        pv = [pspv.tile([128, S], F32, tag="pspv",
                                        name=f"pv{b}_{g}_{j}") for j in range(2)]
                        pt_tiles = {}

                        def scores(kt):
                            vis0 = 128 * (kt - NKC) if kt >= NKC else 0
                            n = S - vis0
                            pt = ptp.tile([128, 2, S], BF16, tag="pt",
                                          name=f"pt{b}_{g}_{kt}")
                            pt_tiles[kt] = pt
                            ps_s = ps.tile([128, 2, S], F32, tag="ps",
                                           name=f"pss{b}_{g}_{kt}")
                            for j in range(2):
                                nc.tensor.matmul(
                                    ps_s[:, j, 0:n],
                                    kT_b[:, 128 * kt:128 * kt + 128],
                                    qT_b[:, 2 * g + j, vis0:S],
                                    start=True, stop=True)
                            if kt < NKC:
                                nc.scalar.activation(
                                    pt[:, :, :], ps_s[:, :, :],
                                    mybir.ActivationFunctionType.Exp,
                                    scale=SCALE, bias=bias_t[:, 0:1])
                            else:
                                te = tep.tile([128, 2, 128], BF16, tag="te")
                                nc.scalar.activation(
                                    te[:, :, :], ps_s[:, :, 0:128],
                                    mybir.ActivationFunctionType.Exp,
                                    scale=SCALE, bias=bias_t[:, 0:1])
                                nc.gpsimd.affine_select(
                                    out=pt[:, :, vis0:vis0 + 128],
                                    in_=te[:, :, :],
                                    pattern=[[0, 2], [1, 128]],
                                    compare_op=mybir.AluOpType.is_ge,
                                    fill=0.0, base=0, channel_multiplier=-1)
                                if n > 128:
                                    nc.scalar.activation(
                                        pt[:, :, vis0 + 128:S],
                                        ps_s[:, :, 128:n],
                                        mybir.ActivationFunctionType.Exp,
                                        scale=SCALE, bias=bias_t[:, 0:1])
                            # denominator accumulation (in-place f32)
                            if kt == 0:
                                nc.vector.tensor_copy(out=den[g][:, :, :],
                                                      in_=pt[:, :, :])
                            else:
                                nc.vector.tensor_tensor(
                                    out=den[g][:, :, vis0:S],
                                    in0=den[g][:, :, vis0:S],
                                    in1=pt[:, :, vis0:S],
                                    op=mybir.AluOpType.add)

                        def pv_step(kt):
                            vis0 = 128 * (kt - NKC) if kt >= NKC else 0
                            pt = pt_tiles.pop(kt)
                            for j in range(2):
                                o = pv[j][:, :] if kt == 0 else pv[j][:, vis0:S]
                                nc.tensor.matmul(o, v_b[:, kt, :],
                                                 pt[:, j, vis0:S],
                                                 start=(kt == 0),
                                                 stop=(kt == KT - 1))

                        for kt in range(KT):
                            scores(kt)
                            if kt >= 1:
                                pv_step(kt - 1)
                        pv_step(KT - 1)

                        # denominator totals (both heads in one ones-matmul),
                        # then normalize + store the gather payload
                        ps_db2 = ps.tile([128, 2, S], F32, tag="ps",
                                         name=f"psdb{b}_{g}")
                        for j in range(2):
                            nc.tensor.matmul(ps_db2[:, j, :], ones_m[:, :],
                                             den[g][:, j, :],
                                             start=True, stop=True)
                        for j in range(2):
                            recb = work.tile([128, S], F32, tag="recb")
                            nc.vector.reciprocal(out=recb[:, :],
                                                 in_=ps_db2[:, j, :])
                            at16 = work.tile([128, S], BF16, tag="at16")
                            nc.vector.tensor_tensor(out=at16[:, :],
                                                    in0=pv[j][:, :],
                                                    in1=recb[:, :],
                                                    op=mybir.AluOpType.mult)
                            at = work.tile([128, S], PAY_DTYPE, tag="at")
                            nc.vector.tensor_copy(out=at[:, :], in_=at16[:, :])
                            h = 2 * g + j
                            nc.gpsimd.dma_start(
                                out=ag_in[b][0][128 * h:128 * h + 128, :],
                                in_=at[:, :])
                        if g == 1:
                            launch_gather(0)

                    def launch_gather(g):
                        if LOCAL_COLL:
                            nc.gpsimd.dma_start(
                                out=ag_out[b][g][:, :].rearrange(
                                    "(c e) q -> c e q", c=NCORES)[0],
                                in_=ag_in[b][g][:, :])
                        else:
                            if COLL_PROBE != "quad":
                                for _d in range(2 if DUP_COLL else 1):
                                    nc.gpsimd.collective_compute(
                                        "AllGather", mybir.AluOpType.bypass,
                                        replica_groups=rg,
                                        ins=[ag_in[b][g][:, :]],
                                        outs=[ag_out[b][g][:, :]])
                            if COLL_PROBE in ("disj", "quad"):
                                nc.gpsimd.collective_compute(
                                    "AllGather", mybir.AluOpType.bypass,
                                    replica_groups=rg_quads,
                                    ins=[ag_in[b][g][:, :]], outs=[dj_out[b][g][:, :]])
                            elif COLL_PROBE == "pair":
                                nc.gpsimd.collective_compute(
                                    "AllGather", mybir.AluOpType.bypass,
                                    replica_groups=rg_pairs,
                                    ins=[ag_in[b][g][:, :]], outs=[dj_out[b][g][:, :]])
                            elif COLL_PROBE == "a2a":
                                nc.gpsimd.collective_compute(
                                    "AllToAll", mybir.AluOpType.bypass,
                                    replica_groups=rg,
                                    ins=[dj_in[b][g][:, :]], outs=[dj_out[b][g][:, :]])

                    for g in range(2):  # two head-pair groups, always
                        attn_group(g)
                        if it == 0 and b == 0 and g == 0:
                            load_wo()
                    if DBG_TAPS and it == 0 and b == 0:
                        nc.sync.dma_start(out=dbg_agin[:, :], in_=ag_in[0][0][:, :])
                        nc.sync.dma_start(out=dbg_agout[:, :], in_=ag_out[0][0][:, :])

                    # ---- output projection, PIPE_DEPTH blocks behind ----
                    pending.append(b)
                    if len(pending) > PIPE_DEPTH:
                        emit_wo(pending.pop(0))
            for pb in pending:
                emit_wo(pb)

    _dedup_ldweights(nc)
    _split_excess_waits(nc)
    return nc


_nc_cache = {}


def _get_nc(iters: int):
    if iters not in _nc_cache:
        _nc_cache[iters] = build_nc(iters)
    return _nc_cache[iters]


def make_in_maps(x, wq, wk, wv, wo, freqs_cos, freqs_sin, cache_k, cache_v):
    bf = lambda a: np.ascontiguousarray(a).astype(NPBF16)
    xT = bf(x.reshape(NT, D).T)
    cosT = bf(freqs_cos.T)
    sinT = bf(freqs_sin.T)
    # permute rope pair dims to [evens, odds] within each head
    wq_p = wq.reshape(H, HD, D)[:, PERM, :].reshape(H * HD, D)
    wk_p = wk.reshape(HKV, HD, D)[:, PERM, :].reshape(HKV * HD, D)
    in_maps = []
    for c in range(NCORES):
        in_maps.append({
            "xT": xT,
            "wqT": bf(wq_p[EL * c:EL * (c + 1), :].T),
            "wkT": bf(wk_p[HD * c:HD * (c + 1), :].T),
            "wvT": bf(wv[HD * c:HD * (c + 1), :].T),
            "woT": bf(wo[EL * c:EL * (c + 1), :].T),
            "ckT": bf(cache_k[:, :, c, :].transpose(0, 2, 1)[:, PERM, :]),
            "cv": bf(cache_v[:, :, c, :]),
            "cosT": cosT, "sinT": sinT,
        })
    return in_maps


def assemble_out(results):
    return np.concatenate(
        [results[c]["out"].T for c in range(NCORES)], axis=1
    ).reshape(B, S, D)


def kernel(x, wq, wk, wv, wo, freqs_cos, freqs_sin, cache_k, cache_v,
           start_pos=START, **_ignored):
    assert x.shape == (B, S, D) and int(start_pos) == START
    nc = _get_nc(1)
    in_maps = make_in_maps(np.asarray(x, np.float32), np.asarray(wq, np.float32),
                           np.asarray(wk, np.float32), np.asarray(wv, np.float32),
                           np.asarray(wo, np.float32),
                           np.asarray(freqs_cos, np.float32),
                           np.asarray(freqs_sin, np.float32),
                           np.asarray(cache_k, np.float32),
                           np.asarray(cache_v, np.float32))
    res = run_bass_kernel_spmd(nc, in_maps, core_ids=list(range(NCORES)),
                               trace=False)
    return assemble_out(res.results)



# revision 34
# speedup vs baseline: 1.0485x; 1.0485x over previous
"""Tensor-parallel attention forward (B=4, S=512, D=4096, H=32, HKV=8, HD=128,
START=512) on 8 TRN2 NeuronCores.

Sharding (chosen): TP over heads. Each core c owns q-heads 4c..4c+3 (wq rows
512c:512c+512), kv-head c (wk/wv rows 128c:128c+128, cache slice c), and
output columns 512c:512c+512 (wo rows 512c:512c+512). x is replicated. After
local attention, per-core attention outputs (head-sharded) are AllGathered
(bf16, split in two per token block for earlier comm start) and each core
computes its own 512-column slice of the output projection — no reduction
collective needed. The host concatenates the 8 column slices.

Host-side layout prep (part of sharding): operands are pre-transposed so the
contraction dim (model dim d / feature dim e) lands on SBUF partitions with
natural-stride DMA, pre-cast to bf16 (the on-device compute precision — this
halves HBM traffic), and RoPE pair dims of wq/wk/cache_k are pre-permuted to
[evens, odds] so the on-chip rotation is two contiguous 64-partition blocks.

Compute: bf16 matmuls (fp32 PSUM accumulate), fp32 softmax denominators.
Causal structure: key-tile kt >= 4 only attends to queries s >= 128*(kt-4);
matmul N, exp and denominator work are trimmed accordingly, and only the
128-wide diagonal block needs the affine predicate fill.
"""
import math

import numpy as np
import ml_dtypes

import concourse.mybir as mybir
from concourse import bass
from concourse.tile import TileContext
from concourse.bass_utils import run_bass_kernel_spmd

F32 = mybir.dt.float32
BF16 = mybir.dt.bfloat16  # compute dtype (fp16 measured slower on PE than bf16)
F8 = mybir.dt.float8e3    # gather payload dtype: e3m4 (4 mantissa bits,
                          # range +-15.5 fits |at|<=8; ~1.3% rms)
PAY_DTYPE = F8            # set to BF16 to bisect fp8 payload issues
NPBF16 = ml_dtypes.bfloat16
PAY_SCALE = 1.5           # 1.5*at: max ~12.1 < 15.5, trims e3m4 subnormal share
EXP_BIAS = -6.0           # exp(s/sqrt(hd) - 6): keeps fp16 pt in range
                          # (max score ~15 -> e^9 ~ 8.1e3 < 65504); the bias
                          # cancels exactly in pv/den

NCORES = 8
B, S, D = 4, 512, 4096
H, HKV, HD = 32, 8, 128
START = 512
T = START + S          # 1024 total kv length
NT = B * S             # 2048 tokens
NH = H // NCORES       # 4 local q heads
EL = NH * HD           # 512 local e width
DT = D // 128          # 32 d-tiles
KT = T // 128          # 8 k-tiles
NKC = START // 128     # 4 cached k-tiles
SCALE = 1.0 / math.sqrt(HD)

# RoPE pair permutation: head-dim reordered to [evens, odds]
PERM = np.concatenate([np.arange(0, HD, 2), np.arange(1, HD, 2)])

SPLIT_AG = True   # kept for compat; NGR is authoritative
NGR = 1           # gathers per token block (fp8 payload: 1 x 2MB slot)
PIPE_DEPTH = 1    # token blocks between a gather and its output projection
PT_BUFS = 2       # probability-tile double/triple buffering
TE_BUFS = 2       # exp-staging tiles for the masked diagonal
DUP_DVE = False   # diagnostic: double rope DVE work
DUP_ACT = False   # diagnostic: double exp work
DUP_POOL = False  # diagnostic: double affine_select work
DUP_COLL = False  # diagnostic: double collectives
COLL_PROBE = None  # diagnostic: None|"disj"|"quad"|"pair"|"a2a" collective probes
LOCAL_COLL = False  # diagnostic (sim): replace AllGather with local dram DMA
DBG_TAPS = False  # diagnostic: dump ag_in/ag_out of block 0
ITER_BARRIER = False  # diagnostic: all-engine barrier between iterations
                      # (steady-state delta then measures single-shot time)

_counter = [0]


def _dedup_ldweights(nc):
    """Drop InstLdweights whose stationary AP is identical to the previous
    PE weight load (weights persist in the PE array across matmuls)."""
    removed = 0
    for f in nc.m.functions:
        for blk in f.blocks:
            last_sig = None
            keep = []
            for inst in blk.instructions:
                tn = type(inst).__name__
                if tn == "InstLdweights":
                    sig = (str(inst.ins[0])
                           + str(getattr(inst, "tile_position", None))
                           + str(getattr(inst, "tile_size", None)))
                    if sig == last_sig and not (inst.sync_info and inst.sync_info.on_wait):
                        removed += 1
                        continue
                    last_sig = sig
                elif tn == "InstMatmult":
                    # f32 matmuls stay self-loading (no split LDW) and
                    # clobber the PE weight array; transpose-mode matmuls
                    # change array state too
                    try:
                        if getattr(inst, "is_transpose", False) or \
                                "float32" in str(inst.ins[1].dtype):
                            last_sig = None
                    except Exception:
                        last_sig = None
                elif getattr(inst, "engine", None) == mybir.EngineType.PE:
                    last_sig = None
                keep.append(inst)
            blk.instructions = keep
    return removed


def _split_excess_waits(nc, cap: int = 1):
    """This walrus build rejects instructions with >1 sync waits; split the
    extras into leading no-ops on the same engine."""
    for f in nc.m.functions:
        for blk in f.blocks:
            insts = blk.instructions
            i = 0
            while i < len(insts):
                inst = insts[i]
                si = inst.sync_info
                if si is not None and si.on_wait is not None and len(si.on_wait) > cap:
                    waits = list(si.on_wait)
                    extra, keep = waits[:-cap], waits[-cap:]
                    nops = []
                    for j in range(0, len(extra), cap):
                        _counter[0] += 1
                        nops.append(mybir.InstNoOp(
                            name=f"waitsplit-{_counter[0]}",
                            engine=inst.engine, ins=[], outs=[],
                            sync_info=mybir.SyncInfo(
                                on_wait=extra[j:j + cap], on_update=[]),
                        ))
                    inst.sync_info = mybir.SyncInfo(
                        on_wait=keep, on_update=list(si.on_update or []))
                    for k, nop in enumerate(nops):
                        insts.insert(i + k, nop)
                    i += len(nops)
                i += 1


def build_nc(iters: int = 1):
    nc = bass.Bass(num_devices=NCORES)

    xT = nc.declare_dram_parameter("xT", [D, NT], BF16, isOutput=False)
    wqT = nc.declare_dram_parameter("wqT", [D, EL], BF16, isOutput=False)
    wkT = nc.declare_dram_parameter("wkT", [D, HD], BF16, isOutput=False)
    wvT = nc.declare_dram_parameter("wvT", [D, HD], BF16, isOutput=False)
    woT = nc.declare_dram_parameter("woT", [D, EL], BF16, isOutput=False)
    ckT = nc.declare_dram_parameter("ckT", [B, HD, START], BF16, isOutput=False)
    cv = nc.declare_dram_parameter("cv", [B, START, HD], BF16, isOutput=False)
    cosT = nc.declare_dram_parameter("cosT", [HD // 2, S], BF16, isOutput=False)
    sinT = nc.declare_dram_parameter("sinT", [HD // 2, S], BF16, isOutput=False)
    out = nc.declare_dram_parameter("out", [EL, NT], F32, isOutput=True)
    if DBG_TAPS:
        dbg_agin = nc.declare_dram_parameter("dbg_agin", [NH * HD, S], PAY_DTYPE,
                                             isOutput=True)
        dbg_agout = nc.declare_dram_parameter("dbg_agout", [NCORES * NH * HD, S],
                                              PAY_DTYPE, isOutput=True)

    ngr = NGR
    hpg = NH // ngr  # heads per gather group
    ag_in = [[nc.dram_tensor(f"ag_in_{b}_{g}", [hpg * HD, S], PAY_DTYPE)
              for g in range(ngr)] for b in range(B)]
    ag_out = [[nc.dram_tensor(f"ag_out_{b}_{g}", [NCORES * hpg * HD, S], PAY_DTYPE,
                              addr_space="Shared") for g in range(ngr)]
              for b in range(B)]

    rg = [list(range(NCORES))]
    rg_quads = [[0, 1, 2, 3], [4, 5, 6, 7]]
    rg_pairs = [[0, 1], [2, 3], [4, 5], [6, 7]]
    if COLL_PROBE in ("disj", "quad"):
        dj_in = [[nc.dram_tensor(f"dj_in_{b}_{g}", [hpg * HD, S], BF16)
                  for g in range(ngr)] for b in range(B)]
        dj_out = [[nc.dram_tensor(f"dj_out_{b}_{g}", [4 * hpg * HD, S], BF16)
                   for g in range(ngr)] for b in range(B)]
    elif COLL_PROBE == "pair":
        dj_in = [[nc.dram_tensor(f"dj_in_{b}_{g}", [hpg * HD, S], BF16)
                  for g in range(ngr)] for b in range(B)]
        dj_out = [[nc.dram_tensor(f"dj_out_{b}_{g}", [2 * hpg * HD, S], BF16)
                   for g in range(ngr)] for b in range(B)]
    elif COLL_PROBE == "a2a":
        dj_in = [[nc.dram_tensor(f"dj_in_{b}_{g}", [8 * hpg * HD, S], BF16)
                  for g in range(ngr)] for b in range(B)]
        dj_out = [[nc.dram_tensor(f"dj_out_{b}_{g}", [8 * hpg * HD, S], BF16)
                   for g in range(ngr)] for b in range(B)]

    with TileContext(nc) as tc:
        with (
            tc.tile_pool(name="wpool", bufs=1) as wpool,
            tc.tile_pool(name="cpool", bufs=1) as cpool,
            tc.tile_pool(name="xpool", bufs=2) as xpool,
            tc.tile_pool(name="qkv", bufs=2) as qkv,
            tc.tile_pool(name="work", bufs=2) as work,
            tc.tile_pool(name="denp", bufs=1) as denp,
            tc.tile_pool(name="ptp", bufs=PT_BUFS) as ptp,
            tc.tile_pool(name="tep", bufs=TE_BUFS) as tep,
            tc.tile_pool(name="rope", bufs=2) as ropep,
            tc.tile_pool(name="gath", bufs=3) as gath,
            tc.tile_pool(name="ps", bufs=2, space="PSUM") as ps,
            tc.tile_pool(name="pspv", bufs=4, space="PSUM") as pspv,
        ):
            # ---- preamble (ordered by first consumption: K/V weights and
            # rope tables first, then wq by head-group column halves)
            wk_s = wpool.tile([128, DT, HD], BF16, tag="wk")
            nc.scalar.dma_start(out=wk_s[:, :, :],
                                in_=wkT[:, :].rearrange("(i p) e -> p i e", p=128))
            wv_s = wpool.tile([128, DT, HD], BF16, tag="wv")
            nc.scalar.dma_start(out=wv_s[:, :, :],
                                in_=wvT[:, :].rearrange("(i p) e -> p i e", p=128))
            cos_s = cpool.tile([64, S], BF16, tag="cos")
            nc.scalar.dma_start(out=cos_s[:, :], in_=cosT[:, :])
            sin_s = cpool.tile([64, S], BF16, tag="sin")
            nc.scalar.dma_start(out=sin_s[:, :], in_=sinT[:, :])
            wq_s = wpool.tile([128, DT, EL], BF16, tag="wq")
            for half in range(2):
                for q in range(2):
                    nc.scalar.dma_start(
                        out=wq_s[:, 16 * q:16 * q + 16,
                                 256 * half:256 * half + 256],
                        in_=wqT[:, 256 * half:256 * half + 256].rearrange(
                            "(i p) e -> p i e", p=128)[:, 16 * q:16 * q + 16, :])
            ones_m = cpool.tile([128, 128], F32, tag="onm")
            nc.vector.memset(ones_m[:, :], 1.0 / PAY_SCALE)
            bias_t = cpool.tile([128, 1], F32, tag="bias")
            nc.vector.memset(bias_t[:, :], EXP_BIAS)
            # wo is not needed until the first output projection — load late
            wo_s = wpool.tile([128, DT, EL], BF16, tag="wo")

            def load_wo():
                for q in range(4):
                    nc.scalar.dma_start(
                        out=wo_s[:, 8 * q:8 * q + 8, :],
                        in_=woT[:, :].rearrange("(i p) e -> p i e", p=128)[:, 8 * q:8 * q + 8, :])

            def rope(dst_a, dst_b, src):
                """dst = rotate(src); src [128, S] PSUM f32 with partitions
                [evens(a) 0:64, odds(b) 64:128]; dst bf16 [64, S] slices."""
                for _dup in range(2 if DUP_DVE else 1):
                    _rope1(dst_a, dst_b, src)

            def _rope1(dst_a, dst_b, src):
                a, bb = src[0:64, :], src[64:128, :]
                t1 = ropep.tile([64, S], F32, tag="rt1")
                t2 = ropep.tile([64, S], F32, tag="rt2")
                nc.vector.tensor_tensor(out=t1[:, :], in0=a, in1=cos_s[:, :],
                                        op=mybir.AluOpType.mult)
                nc.vector.tensor_tensor(out=t2[:, :], in0=bb, in1=sin_s[:, :],
                                        op=mybir.AluOpType.mult)
                nc.vector.tensor_tensor(out=dst_a, in0=t1[:, :], in1=t2[:, :],
                                        op=mybir.AluOpType.subtract)
                t3 = ropep.tile([64, S], F32, tag="rt3")
                t4 = ropep.tile([64, S], F32, tag="rt4")
                nc.vector.tensor_tensor(out=t3[:, :], in0=a, in1=sin_s[:, :],
                                        op=mybir.AluOpType.mult)
                nc.vector.tensor_tensor(out=t4[:, :], in0=bb, in1=cos_s[:, :],
                                        op=mybir.AluOpType.mult)
                nc.vector.tensor_tensor(out=dst_b, in0=t3[:, :], in1=t4[:, :],
                                        op=mybir.AluOpType.add)

            def emit_wo(b):
                """Output projection for block b from the gathers. Gather g's
                tile index i covers e-tile 4*(i//hpg) + hpg*g + i%hpg."""
                ps_y = [pspv.tile([128, S], F32, tag="pspv", name=f"psy{b}_{dj}")
                        for dj in range(4)]
                for g in range(ngr):
                    if COLL_PROBE == "quad":  # timing probe: consume quad AG
                        src = dj_out[b][g][:, :].rearrange("(i p) q -> p i q", p=128)
                    else:
                        src = ag_out[b][g][:, :].rearrange("(i p) q -> p i q", p=128)
                    for c in range(NCORES):  # one chunk per source core
                        # f8e3 moving feeds the PE directly (f16 stationary x
                        # f8 moving matmul is exact on HW, 1 cyc/row)
                        ag_t = gath.tile([128, hpg, S], PAY_DTYPE, tag="agt")
                        cc = c % 4 if COLL_PROBE == "quad" else c
                        nc.gpsimd.dma_start(out=ag_t[:, :, :],
                                            in_=src[:, hpg * cc:hpg * cc + hpg, :])
                        for dj in range(4):
                            for t2 in range(hpg):
                                i = 4 * c + hpg * g + t2
                                nc.tensor.matmul(
                                    ps_y[dj][:, :],
                                    wo_s[:, i, 128 * dj:128 * dj + 128],
                                    ag_t[:, t2, :],
                                    start=(g == 0 and c == 0 and t2 == 0),
                                    stop=(g == ngr - 1 and c == NCORES - 1
                                          and t2 == hpg - 1))
                for dj in range(4):
                    yt = work.tile([128, S], F32, tag="yt")
                    nc.scalar.activation(yt[:, :], ps_y[dj][:, :],
                                         mybir.ActivationFunctionType.Copy,
                                         scale=1.0 / PAY_SCALE)
                    nc.sync.dma_start(
                        out=out[128 * dj:128 * dj + 128, S * b:S * b + S],
                        in_=yt[:, :])

            pending = []
            for it in range(iters):
                if ITER_BARRIER and it > 0:
                    for pb in pending:
                        emit_wo(pb)
                    pending = []
                    nc.all_engine_barrier(sem_only=True)
                for b in range(B):
                    # ---- loads for this token block (= batch b) ----
                    xt0 = xpool.tile([128, DT // 2, S], BF16, tag="xt")
                    xt1 = xpool.tile([128, DT // 2, S], BF16, tag="xt")
                    xsrc = xT[:, S * b:S * b + S].rearrange("(i p) t -> p i t", p=128)
                    for hh in range(2):
                        nc.gpsimd.dma_start(out=xt0[:, 8 * hh:8 * hh + 8, :],
                                            in_=xsrc[:, 8 * hh:8 * hh + 8, :])
                    for hh in range(2):
                        nc.gpsimd.dma_start(out=xt1[:, 8 * hh:8 * hh + 8, :],
                                            in_=xsrc[:, 16 + 8 * hh:16 + 8 * hh + 8, :])

                    def xt(i):
                        return (xt0 if i < DT // 2 else xt1)[:, i % (DT // 2), :]

                    kT_b = qkv.tile([128, T], BF16, tag="kT")
                    nc.gpsimd.dma_start(out=kT_b[:, 0:START], in_=ckT[b])
                    v_b = qkv.tile([128, KT, HD], BF16, tag="v")
                    nc.gpsimd.dma_start(
                        out=v_b[:, 0:NKC, :],
                        in_=cv[b].rearrange("(kt p) dv -> p kt dv", p=128))
                    qT_b = qkv.tile([128, NH, S], BF16, tag="qT")

                    # ---- K and V projections, interleaved per d-tile (x is
                    # consumed at half the matmul rate: block-0 fill is
                    # DMA-paced, not stalled) ----
                    ps_k2 = ps.tile([128, 2, S], F32, tag="ps")
                    ps_k = ps_k2[:, 0, :]
                    ps_vt2 = ps.tile([128, 2, S], F32, tag="ps")
                    ps_vt = ps_vt2[:, 0, :]
                    for i in range(DT):
                        nc.tensor.matmul(ps_k, wk_s[:, i, :], xt(i),
                                         start=(i == 0), stop=(i == DT - 1))
                        nc.tensor.matmul(ps_vt, wv_s[:, i, :], xt(i),
                                         start=(i == 0), stop=(i == DT - 1))
                    rope(kT_b[0:64, START:T], kT_b[64:128, START:T], ps_k)
                    vT = work.tile([128, S], BF16, tag="vT")
                    nc.vector.tensor_copy(out=vT[:, :], in_=ps_vt)
                    for ts in range(S // 128):
                        nc.sync.dma_start(out=v_b[:, NKC + ts, :],
                                          in_=vT[:, 128 * ts:128 * ts + 128],
                                          transpose=True)

                    den = [denp.tile([128, 2, S], F32, tag=f"den{g}",
                                     name=f"den{b}_{g}") for g in range(2)]

                    def attn_group(g):
                        """Q proj + attention + normalize + gather for heads
                        2g, 2g+1 — the group's AllGather launches before the
                        other group's Q projection even starts."""
                        ps_qs = [pspv.tile([128, S], F32, tag="pspv",
                                           name=f"psq{b}_{g}_{j}")
                                 for j in range(2)]
                        for i in range(DT):
                            for j in range(2):
                                h = 2 * g + j
                                nc.tensor.matmul(
                                    ps_qs[j][:, :],
                                    wq_s[:, i, 128 * h:128 * h + 128],
                                    xt(i), start=(i == 0), stop=(i == DT - 1))
                        for j in range(2):
                            h = 2 * g + j
                            rope(qT_b[0:64, h, :], qT_b[64:128, h, :],
                                 ps_qs[j][:, :])

                        pv = [pspv.tile([128, S], F32, tag="pspv",
                                        name=f"pv{b}_{g}_{j}") for j in range(2)]
                        pt_tiles = {}

                        def scores(kt):
                            vis0 = 128 * (kt - NKC) if kt >= NKC else 0
                            n = S - vis0
                            pt = ptp.tile([128, 2, S], BF16, tag="pt",
                                          name=f"pt{b}_{g}_{kt}")
                            pt_tiles[kt] = pt
                            ps_s = ps.tile([128, 2, S], F32, tag="ps",
                                           name=f"pss{b}_{g}_{kt}")
                            for j in range(2):
                                nc.tensor.matmul(
                                    ps_s[:, j, 0:n],
                                    kT_b[:, 128 * kt:128 * kt + 128],
                                    qT_b[:, 2 * g + j, vis0:S],
                                    start=True, stop=True)
                            if kt < NKC:
                                nc.scalar.activation(
                                    pt[:, :, :], ps_s[:, :, :],
                                    mybir.ActivationFunctionType.Exp,
                                    scale=SCALE, bias=bias_t[:, 0:1])
                            else:
                                te = tep.tile([128, 2, 128], BF16, tag="te")
                                nc.scalar.activation(
                                    te[:, :, :], ps_s[:, :, 0:128],
                                    mybir.ActivationFunctionType.Exp,
                                    scale=SCALE, bias=bias_t[:, 0:1])
                                nc.gpsimd.affine_select(
                                    out=pt[:, :, vis0:vis0 + 128],
                                    in_=te[:, :, :],
                                    pattern=[[0, 2], [1, 128]],
                                    compare_op=mybir.AluOpType.is_ge,
                                    fill=0.0, base=0, channel_multiplier=-1)
                                if n > 128:
                                    nc.scalar.activation(
                                        pt[:, :, vis0 + 128:S],
                                        ps_s[:, :, 128:n],
                                        mybir.ActivationFunctionType.Exp,
                                        scale=SCALE, bias=bias_t[:, 0:1])
                            # denominator accumulation (in-place f32)
                            if kt == 0:
                                nc.vector.tensor_copy(out=den[g][:, :, :],
                                                      in_=pt[:, :, :])
                            else:
                                nc.vector.tensor_tensor(
                                    out=den[g][:, :, vis0:S],
                                    in0=den[g][:, :, vis0:S],
                                    in1=pt[:, :, vis0:S],
                                    op=mybir.AluOpType.add)

                        def pv_step(kt):
                            vis0 = 128 * (kt - NKC) if kt >= NKC else 0
                            pt = pt_tiles.pop(kt)
                            for j in range(2):
                                o = pv[j][:, :] if kt == 0 else pv[j][:, vis0:S]
                                nc.tensor.matmul(o, v_b[:, kt, :],
                                                 pt[:, j, vis0:S],
                                                 start=(kt == 0),
                                                 stop=(kt == KT - 1))

                        for kt in range(KT):
                            scores(kt)
                            if kt >= 1:
                                pv_step(kt - 1)
                        pv_step(KT - 1)

                        # denominator totals (both heads in one ones-matmul),
                        # then normalize + store the gather payload
                        ps_db2 = ps.tile([128, 2, S], F32, tag="ps",
                                         name=f"psdb{b}_{g}")
                        for j in range(2):
                            nc.tensor.matmul(ps_db2[:, j, :], ones_m[:, :],
                                             den[g][:, j, :],
                                             start=True, stop=True)
                        for j in range(2):
                            recb = work.tile([128, S], F32, tag="recb")
                            nc.vector.reciprocal(out=recb[:, :],
                                                 in_=ps_db2[:, j, :])
                            at16 = work.tile([128, S], BF16, tag="at16")
                            nc.vector.tensor_tensor(out=at16[:, :],
                                                    in0=pv[j][:, :],
                                                    in1=recb[:, :],
                                                    op=mybir.AluOpType.mult)
                            at = work.tile([128, S], PAY_DTYPE, tag="at")
                            nc.vector.tensor_copy(out=at[:, :], in_=at16[:, :])
                            h = 2 * g + j
                            nc.gpsimd.dma_start(
                                out=ag_in[b][0][128 * h:128 * h + 128, :],
                                in_=at[:, :])
                        if g == 1:
                            launch_gather(0)

                    def launch_gather(g):
                        if LOCAL_COLL:
                            nc.gpsimd.dma_start(
                                out=ag_out[b][g][:, :].rearrange(
                                    "(c e) q -> c e q", c=NCORES)[0],
                                in_=ag_in[b][g][:, :])
                        else:
                            if COLL_PROBE != "quad":
                                for _d in range(2 if DUP_COLL else 1):
                                    nc.gpsimd.collective_compute(
                                        "AllGather", mybir.AluOpType.bypass,
                                        replica_groups=rg,
                                        ins=[ag_in[b][g][:, :]],
                                        outs=[ag_out[b][g][:, :]])
                            if COLL_PROBE in ("disj", "quad"):
                                nc.gpsimd.collective_compute(
                                    "AllGather", mybir.AluOpType.bypass,
                                    replica_groups=rg_quads,
                                    ins=[ag_in[b][g][:, :]], outs=[dj_out[b][g][:, :]])
                            elif COLL_PROBE == "pair":
                                nc.gpsimd.collective_compute(
                                    "AllGather", mybir.AluOpType.bypass,
                                    replica_groups=rg_pairs,
                                    ins=[ag_in[b][g][:, :]], outs=[dj_out[b][g][:, :]])
                            elif COLL_PROBE == "a2a":
                                nc.gpsimd.collective_compute(
                                    "AllToAll", mybir.AluOpType.bypass,
                                    replica_groups=rg,
                                    ins=[dj_in[b][g][:, :]], outs=[dj_out[b][g][:, :]])

                    for g in range(2):  # two head-pair groups, always
                        attn_group(g)
                        if it == 0 and b == 0 and g == 0:
                            load_wo()
                    if DBG_TAPS and it == 0 and b == 0:
                        nc.sync.dma_start(out=dbg_agin[:, :], in_=ag_in[0][0][:, :])
                        nc.sync.dma_start(out=dbg_agout[:, :], in_=ag_out[0][0][:, :])

                    # ---- output projection, PIPE_DEPTH blocks behind ----
                    pending.append(b)
                    if len(pending) > PIPE_DEPTH:
                        emit_wo(pending.pop(0))
            for pb in pending:
                emit_wo(pb)

    _dedup_ldweights(nc)
    _split_excess_waits(nc)
    return nc


_nc_cache = {}


def _get_nc(iters: int):
    if iters not in _nc_cache:
        _nc_cache[iters] = build_nc(iters)
    return _nc_cache[iters]


def make_in_maps(x, wq, wk, wv, wo, freqs_cos, freqs_sin, cache_k, cache_v):
    bf = lambda a: np.ascontiguousarray(a).astype(NPBF16)
    xT = bf(x.reshape(NT, D).T)
    cosT = bf(freqs_cos.T)
    sinT = bf(freqs_sin.T)
    # permute rope pair dims to [evens, odds] within each head
    wq_p = wq.reshape(H, HD, D)[:, PERM, :].reshape(H * HD, D)
    wk_p = wk.reshape(HKV, HD, D)[:, PERM, :].reshape(HKV * HD, D)
    in_maps = []
    for c in range(NCORES):
        in_maps.append({
            "xT": xT,
            "wqT": bf(wq_p[EL * c:EL * (c + 1), :].T),
            "wkT": bf(wk_p[HD * c:HD * (c + 1), :].T),
            "wvT": bf(wv[HD * c:HD * (c + 1), :].T),
            "woT": bf(wo[EL * c:EL * (c + 1), :].T),
            "ckT": bf(cache_k[:, :, c, :].transpose(0, 2, 1)[:, PERM, :]),
            "cv": bf(cache_v[:, :, c, :]),
            "cosT": cosT, "sinT": sinT,
        })
    return in_maps


def assemble_out(results):
    return np.concatenate(
        [results[c]["out"].T for c in range(NCORES)], axis=1
    ).reshape(B, S, D)


def kernel(x, wq, wk, wv, wo, freqs_cos, freqs_sin, cache_k, cache_v,
           start_pos=START, **_ignored):
    assert x.shape == (B, S, D) and int(start_pos) == START
    nc = _get_nc(1)
    in_maps = make_in_maps(np.asarray(x, np.float32), np.asarray(wq, np.float32),
                           np.asarray(wk, np.float32), np.asarray(wv, np.float32),
                           np.asarray(wo, np.float32),
                           np.asarray(freqs_cos, np.float32),
                           np.asarray(freqs_sin, np.float32),
                           np.asarray(cache_k, np.float32),
                           np.asarray(cache_v, np.float32))
    res = run_bass_kernel_spmd(nc, in_maps, core_ids=list(range(NCORES)),
                               trace=False)
    return assemble_out(res.results)



# revision 39
# speedup vs baseline: 1.1184x; 1.0667x over previous
"""Tensor-parallel attention forward (B=4, S=512, D=4096, H=32, HKV=8, HD=128,
START=512) on 8 TRN2 NeuronCores.

Sharding (chosen): TP over heads. Each core c owns q-heads 4c..4c+3 (wq rows
512c:512c+512), kv-head c (wk/wv rows 128c:128c+128, cache slice c), and
output columns 512c:512c+512 (wo rows 512c:512c+512). x is replicated. After
local attention, per-core attention outputs (head-sharded) are AllGathered
(bf16, split in two per token block for earlier comm start) and each core
computes its own 512-column slice of the output projection — no reduction
collective needed. The host concatenates the 8 column slices.

Host-side layout prep (part of sharding): operands are pre-transposed so the
contraction dim (model dim d / feature dim e) lands on SBUF partitions with
natural-stride DMA, pre-cast to bf16 (the on-device compute precision — this
halves HBM traffic), and RoPE pair dims of wq/wk/cache_k are pre-permuted to
[evens, odds] so the on-chip rotation is two contiguous 64-partition blocks.

Compute: bf16 matmuls (fp32 PSUM accumulate), fp32 softmax denominators.
Causal structure: key-tile kt >= 4 only attends to queries s >= 128*(kt-4);
matmul N, exp and denominator work are trimmed accordingly, and only the
128-wide diagonal block needs the affine predicate fill.
"""
import math

import numpy as np
import ml_dtypes

import concourse.mybir as mybir
from concourse import bass
from concourse.tile import TileContext
from concourse.bass_utils import run_bass_kernel_spmd

F32 = mybir.dt.float32
BF16 = mybir.dt.bfloat16  # compute dtype (fp16 measured slower on PE than bf16)
F8 = mybir.dt.float8e3    # gather payload dtype: e3m4 (4 mantissa bits,
                          # range +-15.5 fits |at|<=8; ~1.3% rms)
PAY_DTYPE = F8            # set to BF16 to bisect fp8 payload issues
NPBF16 = ml_dtypes.bfloat16
PAY_SCALE = 1.5           # 1.5*at: max ~12.1 < 15.5, trims e3m4 subnormal share
EXP_BIAS = -6.0           # exp(s/sqrt(hd) - 6): keeps fp16 pt in range
                          # (max score ~15 -> e^9 ~ 8.1e3 < 65504); the bias
                          # cancels exactly in pv/den

NCORES = 8
B, S, D = 4, 512, 4096
H, HKV, HD = 32, 8, 128
START = 512
T = START + S          # 1024 total kv length
NT = B * S             # 2048 tokens
NH = H // NCORES       # 4 local q heads
EL = NH * HD           # 512 local e width
DT = D // 128          # 32 d-tiles
KT = T // 128          # 8 k-tiles
NKC = START // 128     # 4 cached k-tiles
SCALE = 1.0 / math.sqrt(HD)

# RoPE pair permutation: head-dim reordered to [evens, odds]
PERM = np.concatenate([np.arange(0, HD, 2), np.arange(1, HD, 2)])

SPLIT_AG = True   # kept for compat; NGR is authoritative
NGR = 1           # gathers per token block (fp8 payload: 1 x 2MB slot)
PIPE_DEPTH = 1    # token blocks between a gather and its output projection
PT_BUFS = 2       # probability-tile double/triple buffering
TE_BUFS = 2       # exp-staging tiles for the masked diagonal
DUP_DVE = False   # diagnostic: double rope DVE work
DUP_ACT = False   # diagnostic: double exp work
DUP_POOL = False  # diagnostic: double affine_select work
DUP_COLL = False  # diagnostic: double collectives
COLL_PROBE = None  # diagnostic: None|"disj"|"quad"|"pair"|"a2a" collective probes
LOCAL_COLL = False  # diagnostic (sim): replace AllGather with local dram DMA
DBG_TAPS = False  # diagnostic: dump ag_in/ag_out of block 0
DEQUANT_DVE = False  # dequant payload on DVE instead of f8-moving matmul
ITER_BARRIER = False  # diagnostic: all-engine barrier between iterations
                      # (steady-state delta then measures single-shot time)

_counter = [0]


def _dedup_ldweights(nc):
    """Drop InstLdweights whose stationary AP is identical to the previous
    PE weight load (weights persist in the PE array across matmuls)."""
    removed = 0
    for f in nc.m.functions:
        for blk in f.blocks:
            last_sig = None
            keep = []
            for inst in blk.instructions:
                tn = type(inst).__name__
                if tn == "InstLdweights":
                    sig = (str(inst.ins[0])
                           + str(getattr(inst, "tile_position", None))
                           + str(getattr(inst, "tile_size", None)))
                    if sig == last_sig and not (inst.sync_info and inst.sync_info.on_wait):
                        removed += 1
                        continue
                    last_sig = sig
                elif tn == "InstMatmult":
                    # f32 matmuls stay self-loading (no split LDW) and
                    # clobber the PE weight array; transpose-mode matmuls
                    # change array state too
                    try:
                        if getattr(inst, "is_transpose", False) or \
                                "float32" in str(inst.ins[1].dtype):
                            last_sig = None
                    except Exception:
                        last_sig = None
                elif getattr(inst, "engine", None) == mybir.EngineType.PE:
                    last_sig = None
                keep.append(inst)
            blk.instructions = keep
    return removed


def _split_excess_waits(nc, cap: int = 1):
    """This walrus build rejects instructions with >1 sync waits; split the
    extras into leading no-ops on the same engine."""
    for f in nc.m.functions:
        for blk in f.blocks:
            insts = blk.instructions
            i = 0
            while i < len(insts):
                inst = insts[i]
                si = inst.sync_info
                if si is not None and si.on_wait is not None and len(si.on_wait) > cap:
                    waits = list(si.on_wait)
                    extra, keep = waits[:-cap], waits[-cap:]
                    nops = []
                    for j in range(0, len(extra), cap):
                        _counter[0] += 1
                        nops.append(mybir.InstNoOp(
                            name=f"waitsplit-{_counter[0]}",
                            engine=inst.engine, ins=[], outs=[],
                            sync_info=mybir.SyncInfo(
                                on_wait=extra[j:j + cap], on_update=[]),
                        ))
                    inst.sync_info = mybir.SyncInfo(
                        on_wait=keep, on_update=list(si.on_update or []))
                    for k, nop in enumerate(nops):
                        insts.insert(i + k, nop)
                    i += len(nops)
                i += 1


def build_nc(iters: int = 1):
    nc = bass.Bass(num_devices=NCORES)

    xT = nc.declare_dram_parameter("xT", [D, NT], BF16, isOutput=False)
    wqT = nc.declare_dram_parameter("wqT", [D, EL], BF16, isOutput=False)
    wkT = nc.declare_dram_parameter("wkT", [D, HD], BF16, isOutput=False)
    wvT = nc.declare_dram_parameter("wvT", [D, HD], BF16, isOutput=False)
    woT = nc.declare_dram_parameter("woT", [D, EL], BF16, isOutput=False)
    ckT = nc.declare_dram_parameter("ckT", [B, HD, START], BF16, isOutput=False)
    cv = nc.declare_dram_parameter("cv", [B, START, HD], BF16, isOutput=False)
    cosT = nc.declare_dram_parameter("cosT", [HD // 2, S], BF16, isOutput=False)
    sinT = nc.declare_dram_parameter("sinT", [HD // 2, S], BF16, isOutput=False)
    out = nc.declare_dram_parameter("out", [EL, NT], F32, isOutput=True)
    if DBG_TAPS:
        dbg_agin = nc.declare_dram_parameter("dbg_agin", [NH * HD, S], PAY_DTYPE,
                                             isOutput=True)
        dbg_agout = nc.declare_dram_parameter("dbg_agout", [NCORES * NH * HD, S],
                                              PAY_DTYPE, isOutput=True)

    ngr = NGR
    hpg = NH // ngr  # heads per gather group
    ag_in = [[nc.dram_tensor(f"ag_in_{b}_{g}", [hpg * HD, S], PAY_DTYPE)
              for g in range(ngr)] for b in range(B)]
    ag_out = [[nc.dram_tensor(f"ag_out_{b}_{g}", [NCORES * hpg * HD, S], PAY_DTYPE,
                              addr_space="Shared") for g in range(ngr)]
              for b in range(B)]

    rg = [list(range(NCORES))]
    rg_quads = [[0, 1, 2, 3], [4, 5, 6, 7]]
    rg_pairs = [[0, 1], [2, 3], [4, 5], [6, 7]]
    if COLL_PROBE in ("disj", "quad"):
        dj_in = [[nc.dram_tensor(f"dj_in_{b}_{g}", [hpg * HD, S], BF16)
                  for g in range(ngr)] for b in range(B)]
        dj_out = [[nc.dram_tensor(f"dj_out_{b}_{g}", [4 * hpg * HD, S], BF16)
                   for g in range(ngr)] for b in range(B)]
    elif COLL_PROBE == "pair":
        dj_in = [[nc.dram_tensor(f"dj_in_{b}_{g}", [hpg * HD, S], BF16)
                  for g in range(ngr)] for b in range(B)]
        dj_out = [[nc.dram_tensor(f"dj_out_{b}_{g}", [2 * hpg * HD, S], BF16)
                   for g in range(ngr)] for b in range(B)]
    elif COLL_PROBE == "a2a":
        dj_in = [[nc.dram_tensor(f"dj_in_{b}_{g}", [8 * hpg * HD, S], BF16)
                  for g in range(ngr)] for b in range(B)]
        dj_out = [[nc.dram_tensor(f"dj_out_{b}_{g}", [8 * hpg * HD, S], BF16)
                   for g in range(ngr)] for b in range(B)]

    with TileContext(nc) as tc:
        with (
            tc.tile_pool(name="wpool", bufs=1) as wpool,
            tc.tile_pool(name="cpool", bufs=1) as cpool,
            tc.tile_pool(name="xpool", bufs=2) as xpool,
            tc.tile_pool(name="qkv", bufs=2) as qkv,
            tc.tile_pool(name="work", bufs=2) as work,
            tc.tile_pool(name="denp", bufs=1) as denp,
            tc.tile_pool(name="ptp", bufs=PT_BUFS) as ptp,
            tc.tile_pool(name="tep", bufs=TE_BUFS) as tep,
            tc.tile_pool(name="rope", bufs=2) as ropep,
            tc.tile_pool(name="gath", bufs=3) as gath,
            tc.tile_pool(name="ps", bufs=2, space="PSUM") as ps,
            tc.tile_pool(name="pspv", bufs=4, space="PSUM") as pspv,
        ):
            # ---- preamble (ordered by first consumption: K/V weights and
            # rope tables first, then wq by head-group column halves)
            wk_s = wpool.tile([128, DT, HD], BF16, tag="wk")
            nc.scalar.dma_start(out=wk_s[:, :, :],
                                in_=wkT[:, :].rearrange("(i p) e -> p i e", p=128))
            wv_s = wpool.tile([128, DT, HD], BF16, tag="wv")
            nc.scalar.dma_start(out=wv_s[:, :, :],
                                in_=wvT[:, :].rearrange("(i p) e -> p i e", p=128))
            cos_s = cpool.tile([64, S], BF16, tag="cos")
            nc.scalar.dma_start(out=cos_s[:, :], in_=cosT[:, :])
            sin_s = cpool.tile([64, S], BF16, tag="sin")
            nc.scalar.dma_start(out=sin_s[:, :], in_=sinT[:, :])
            wq_s = wpool.tile([128, DT, EL], BF16, tag="wq")
            for half in range(2):
                for q in range(2):
                    nc.scalar.dma_start(
                        out=wq_s[:, 16 * q:16 * q + 16,
                                 256 * half:256 * half + 256],
                        in_=wqT[:, 256 * half:256 * half + 256].rearrange(
                            "(i p) e -> p i e", p=128)[:, 16 * q:16 * q + 16, :])
            ones_m = cpool.tile([128, 128], F32, tag="onm")
            nc.vector.memset(ones_m[:, :], 1.0 / PAY_SCALE)
            bias_t = cpool.tile([128, 1], F32, tag="bias")
            nc.vector.memset(bias_t[:, :], EXP_BIAS)
            # wo is not needed until the first output projection — load late
            wo_s = wpool.tile([128, DT, EL], BF16, tag="wo")

            def load_wo():
                for q in range(4):
                    nc.scalar.dma_start(
                        out=wo_s[:, 8 * q:8 * q + 8, :],
                        in_=woT[:, :].rearrange("(i p) e -> p i e", p=128)[:, 8 * q:8 * q + 8, :])

            def rope(dst_a, dst_b, src):
                """dst = rotate(src); src [128, S] PSUM f32 with partitions
                [evens(a) 0:64, odds(b) 64:128]; dst bf16 [64, S] slices."""
                for _dup in range(2 if DUP_DVE else 1):
                    _rope1(dst_a, dst_b, src)

            def _rope1(dst_a, dst_b, src):
                a, bb = src[0:64, :], src[64:128, :]
                t1 = ropep.tile([64, S], F32, tag="rt1")
                t2 = ropep.tile([64, S], F32, tag="rt2")
                nc.vector.tensor_tensor(out=t1[:, :], in0=a, in1=cos_s[:, :],
                                        op=mybir.AluOpType.mult)
                nc.vector.tensor_tensor(out=t2[:, :], in0=bb, in1=sin_s[:, :],
                                        op=mybir.AluOpType.mult)
                nc.vector.tensor_tensor(out=dst_a, in0=t1[:, :], in1=t2[:, :],
                                        op=mybir.AluOpType.subtract)
                t3 = ropep.tile([64, S], F32, tag="rt3")
                t4 = ropep.tile([64, S], F32, tag="rt4")
                nc.vector.tensor_tensor(out=t3[:, :], in0=a, in1=sin_s[:, :],
                                        op=mybir.AluOpType.mult)
                nc.vector.tensor_tensor(out=t4[:, :], in0=bb, in1=cos_s[:, :],
                                        op=mybir.AluOpType.mult)
                nc.vector.tensor_tensor(out=dst_b, in0=t3[:, :], in1=t4[:, :],
                                        op=mybir.AluOpType.add)

            def emit_wo(b):
                """Output projection for block b from the gathers. Gather g's
                tile index i covers e-tile 4*(i//hpg) + hpg*g + i%hpg."""
                ps_y = [pspv.tile([128, S], F32, tag="pspv", name=f"psy{b}_{dj}")
                        for dj in range(4)]
                for g in range(ngr):
                    if COLL_PROBE == "quad":  # timing probe: consume quad AG
                        src = dj_out[b][g][:, :].rearrange("(i p) q -> p i q", p=128)
                    else:
                        src = ag_out[b][g][:, :].rearrange("(i p) q -> p i q", p=128)
                    for c in range(NCORES):  # one chunk per source core
                        cc = c % 4 if COLL_PROBE == "quad" else c
                        if DEQUANT_DVE:
                            ag_t8 = gath.tile([128, hpg, S], PAY_DTYPE, tag="agt8")
                            nc.gpsimd.dma_start(
                                out=ag_t8[:, :, :],
                                in_=src[:, hpg * cc:hpg * cc + hpg, :])
                            ag_t = gath.tile([128, hpg, S], BF16, tag="agt")
                            nc.vector.tensor_copy(out=ag_t[:, :, :],
                                                  in_=ag_t8[:, :, :])
                        else:
                            # f8e3 moving feeds the PE directly (exact on HW)
                            ag_t = gath.tile([128, hpg, S], PAY_DTYPE, tag="agt")
                            nc.gpsimd.dma_start(
                                out=ag_t[:, :, :],
                                in_=src[:, hpg * cc:hpg * cc + hpg, :])
                        for dj in range(4):
                            for t2 in range(hpg):
                                i = 4 * c + hpg * g + t2
                                nc.tensor.matmul(
                                    ps_y[dj][:, :],
                                    wo_s[:, i, 128 * dj:128 * dj + 128],
                                    ag_t[:, t2, :],
                                    start=(g == 0 and c == 0 and t2 == 0),
                                    stop=(g == ngr - 1 and c == NCORES - 1
                                          and t2 == hpg - 1))
                for dj in range(4):
                    yt = work.tile([128, S], F32, tag="yt")
                    nc.scalar.activation(yt[:, :], ps_y[dj][:, :],
                                         mybir.ActivationFunctionType.Copy,
                                         scale=1.0 / PAY_SCALE)
                    nc.sync.dma_start(
                        out=out[128 * dj:128 * dj + 128, S * b:S * b + S],
                        in_=yt[:, :])

            pending = []
            for it in range(iters):
                if ITER_BARRIER and it > 0:
                    for pb in pending:
                        emit_wo(pb)
                    pending = []
                    nc.all_engine_barrier(sem_only=True)
                def emit_loads(b):
                    # input loads for token block b (= batch b); hoisted a
                    # block early so the gpsimd queue's late-block ops (at
                    # stores, gather) never head-of-line-block the prefetch
                    xt0 = xpool.tile([128, DT // 2, S], BF16, tag="xt")
                    xt1 = xpool.tile([128, DT // 2, S], BF16, tag="xt")
                    xsrc = xT[:, S * b:S * b + S].rearrange("(i p) t -> p i t", p=128)
                    for hh in range(2):
                        nc.gpsimd.dma_start(out=xt0[:, 8 * hh:8 * hh + 8, :],
                                            in_=xsrc[:, 8 * hh:8 * hh + 8, :])
                    for hh in range(2):
                        nc.gpsimd.dma_start(out=xt1[:, 8 * hh:8 * hh + 8, :],
                                            in_=xsrc[:, 16 + 8 * hh:16 + 8 * hh + 8, :])
                    kT_b = qkv.tile([128, T], BF16, tag="kT")
                    nc.gpsimd.dma_start(out=kT_b[:, 0:START], in_=ckT[b])
                    v_b = qkv.tile([128, KT, HD], BF16, tag="v")
                    nc.gpsimd.dma_start(
                        out=v_b[:, 0:NKC, :],
                        in_=cv[b].rearrange("(kt p) dv -> p kt dv", p=128))
                    return xt0, xt1, kT_b, v_b

                if it == 0 or ITER_BARRIER:
                    loads = emit_loads(0)
                for b in range(B):
                    xt0, xt1, kT_b, v_b = loads

                    def xt(i):
                        return (xt0 if i < DT // 2 else xt1)[:, i % (DT // 2), :]

                    qT_b = qkv.tile([128, NH, S], BF16, tag="qT")

                    # ---- K and V projections, interleaved per d-tile (x is
                    # consumed at half the matmul rate: block-0 fill is
                    # DMA-paced, not stalled) ----
                    ps_k2 = ps.tile([128, 2, S], F32, tag="ps")
                    ps_k = ps_k2[:, 0, :]
                    ps_vt2 = ps.tile([128, 2, S], F32, tag="ps")
                    ps_vt = ps_vt2[:, 0, :]
                    for i in range(DT):
                        nc.tensor.matmul(ps_k, wk_s[:, i, :], xt(i),
                                         start=(i == 0), stop=(i == DT - 1))
                        nc.tensor.matmul(ps_vt, wv_s[:, i, :], xt(i),
                                         start=(i == 0), stop=(i == DT - 1))
                    rope(kT_b[0:64, START:T], kT_b[64:128, START:T], ps_k)
                    vT = work.tile([128, S], BF16, tag="vT")
                    nc.vector.tensor_copy(out=vT[:, :], in_=ps_vt)
                    for ts in range(S // 128):
                        nc.sync.dma_start(out=v_b[:, NKC + ts, :],
                                          in_=vT[:, 128 * ts:128 * ts + 128],
                                          transpose=True)

                    den = [denp.tile([128, 2, S], F32, tag=f"den{g}",
                                     name=f"den{b}_{g}") for g in range(2)]

                    def qproj_group(g):
                        """Q projection + rope for heads 2g, 2g+1 (d-tile
                        outer).  Issued for both groups back to back so group
                        g0's rope (DVE) overlaps group g1's matmuls (PE)."""
                        ps_qs = [pspv.tile([128, S], F32, tag="pspv",
                                           name=f"psq{b}_{g}_{j}")
                                 for j in range(2)]
                        for i in range(DT):
                            for j in range(2):
                                h = 2 * g + j
                                nc.tensor.matmul(
                                    ps_qs[j][:, :],
                                    wq_s[:, i, 128 * h:128 * h + 128],
                                    xt(i), start=(i == 0), stop=(i == DT - 1))
                        for j in range(2):
                            h = 2 * g + j
                            rope(qT_b[0:64, h, :], qT_b[64:128, h, :],
                                 ps_qs[j][:, :])

                    def attn_group(g):
                        """Attention + normalize + gather for heads 2g,
                        2g+1."""
                        pv = [pspv.tile([128, S], F32, tag="pspv",
                                        name=f"pv{b}_{g}_{j}") for j in range(2)]
                        pt_tiles = {}

                        def scores(kt):
                            vis0 = 128 * (kt - NKC) if kt >= NKC else 0
                            n = S - vis0
                            pt = ptp.tile([128, 2, S], BF16, tag="pt",
                                          name=f"pt{b}_{g}_{kt}")
                            pt_tiles[kt] = pt
                            ps_s = ps.tile([128, 2, S], F32, tag="ps",
                                           name=f"pss{b}_{g}_{kt}")
                            for j in range(2):
                                nc.tensor.matmul(
                                    ps_s[:, j, 0:n],
                                    kT_b[:, 128 * kt:128 * kt + 128],
                                    qT_b[:, 2 * g + j, vis0:S],
                                    start=True, stop=True)
                            if kt < NKC:
                                nc.scalar.activation(
                                    pt[:, :, :], ps_s[:, :, :],
                                    mybir.ActivationFunctionType.Exp,
                                    scale=SCALE, bias=bias_t[:, 0:1])
                            else:
                                te = tep.tile([128, 2, 128], BF16, tag="te")
                                nc.scalar.activation(
                                    te[:, :, :], ps_s[:, :, 0:128],
                                    mybir.ActivationFunctionType.Exp,
                                    scale=SCALE, bias=bias_t[:, 0:1])
                                nc.gpsimd.affine_select(
                                    out=pt[:, :, vis0:vis0 + 128],
                                    in_=te[:, :, :],
                                    pattern=[[0, 2], [1, 128]],
                                    compare_op=mybir.AluOpType.is_ge,
                                    fill=0.0, base=0, channel_multiplier=-1)
                                if n > 128:
                                    nc.scalar.activation(
                                        pt[:, :, vis0 + 128:S],
                                        ps_s[:, :, 128:n],
                                        mybir.ActivationFunctionType.Exp,
                                        scale=SCALE, bias=bias_t[:, 0:1])
                            # denominator accumulation (in-place f32)
                            if kt == 0:
                                nc.vector.tensor_copy(out=den[g][:, :, :],
                                                      in_=pt[:, :, :])
                            else:
                                nc.vector.tensor_tensor(
                                    out=den[g][:, :, vis0:S],
                                    in0=den[g][:, :, vis0:S],
                                    in1=pt[:, :, vis0:S],
                                    op=mybir.AluOpType.add)

                        def pv_step(kt):
                            vis0 = 128 * (kt - NKC) if kt >= NKC else 0
                            pt = pt_tiles.pop(kt)
                            for j in range(2):
                                o = pv[j][:, :] if kt == 0 else pv[j][:, vis0:S]
                                nc.tensor.matmul(o, v_b[:, kt, :],
                                                 pt[:, j, vis0:S],
                                                 start=(kt == 0),
                                                 stop=(kt == KT - 1))

                        for kt in range(KT):
                            scores(kt)
                            if kt >= 1:
                                pv_step(kt - 1)
                        pv_step(KT - 1)

                        # denominator totals (both heads in one ones-matmul),
                        # then normalize + store the gather payload
                        ps_db2 = ps.tile([128, 2, S], F32, tag="ps",
                                         name=f"psdb{b}_{g}")
                        for j in range(2):
                            nc.tensor.matmul(ps_db2[:, j, :], ones_m[:, :],
                                             den[g][:, j, :],
                                             start=True, stop=True)
                        for j in range(2):
                            recb = work.tile([128, S], F32, tag="recb")
                            nc.vector.reciprocal(out=recb[:, :],
                                                 in_=ps_db2[:, j, :])
                            at16 = work.tile([128, S], BF16, tag="at16")
                            nc.vector.tensor_tensor(out=at16[:, :],
                                                    in0=pv[j][:, :],
                                                    in1=recb[:, :],
                                                    op=mybir.AluOpType.mult)
                            at = work.tile([128, S], PAY_DTYPE, tag="at")
                            nc.vector.tensor_copy(out=at[:, :], in_=at16[:, :])
                            h = 2 * g + j
                            nc.gpsimd.dma_start(
                                out=ag_in[b][0][128 * h:128 * h + 128, :],
                                in_=at[:, :])
                        if g == 1:
                            launch_gather(0)

                    def launch_gather(g):
                        if LOCAL_COLL:
                            nc.gpsimd.dma_start(
                                out=ag_out[b][g][:, :].rearrange(
                                    "(c e) q -> c e q", c=NCORES)[0],
                                in_=ag_in[b][g][:, :])
                        else:
                            if COLL_PROBE != "quad":
                                for _d in range(2 if DUP_COLL else 1):
                                    nc.gpsimd.collective_compute(
                                        "AllGather", mybir.AluOpType.bypass,
                                        replica_groups=rg,
                                        ins=[ag_in[b][g][:, :]],
                                        outs=[ag_out[b][g][:, :]])
                            if COLL_PROBE in ("disj", "quad"):
                                nc.gpsimd.collective_compute(
                                    "AllGather", mybir.AluOpType.bypass,
                                    replica_groups=rg_quads,
                                    ins=[ag_in[b][g][:, :]], outs=[dj_out[b][g][:, :]])
                            elif COLL_PROBE == "pair":
                                nc.gpsimd.collective_compute(
                                    "AllGather", mybir.AluOpType.bypass,
                                    replica_groups=rg_pairs,
                                    ins=[ag_in[b][g][:, :]], outs=[dj_out[b][g][:, :]])
                            elif COLL_PROBE == "a2a":
                                nc.gpsimd.collective_compute(
                                    "AllToAll", mybir.AluOpType.bypass,
                                    replica_groups=rg,
                                    ins=[dj_in[b][g][:, :]], outs=[dj_out[b][g][:, :]])

                    qproj_group(0)
                    if it == 0 and b == 0:
                        load_wo()
                    qproj_group(1)
                    if b + 1 < B:
                        loads = emit_loads(b + 1)
                    elif it + 1 < iters and not ITER_BARRIER:
                        loads = emit_loads(0)  # next iteration's first block
                    for g in range(2):  # two head-pair groups, always
                        attn_group(g)
                    if DBG_TAPS and it == 0 and b == 0:
                        nc.sync.dma_start(out=dbg_agin[:, :], in_=ag_in[0][0][:, :])
                        nc.sync.dma_start(out=dbg_agout[:, :], in_=ag_out[0][0][:, :])

                    # ---- output projection, PIPE_DEPTH blocks behind ----
                    pending.append(b)
                    if len(pending) > PIPE_DEPTH:
                        emit_wo(pending.pop(0))
            for pb in pending:
                emit_wo(pb)

    _dedup_ldweights(nc)
    _split_excess_waits(nc)
    return nc


_nc_cache = {}


def _get_nc(iters: int):
    if iters not in _nc_cache:
        _nc_cache[iters] = build_nc(iters)
    return _nc_cache[iters]


def make_in_maps(x, wq, wk, wv, wo, freqs_cos, freqs_sin, cache_k, cache_v):
    bf = lambda a: np.ascontiguousarray(a).astype(NPBF16)
    xT = bf(x.reshape(NT, D).T)
    cosT = bf(freqs_cos.T)
    sinT = bf(freqs_sin.T)
    # permute rope pair dims to [evens, odds] within each head
    wq_p = wq.reshape(H, HD, D)[:, PERM, :].reshape(H * HD, D)
    wk_p = wk.reshape(HKV, HD, D)[:, PERM, :].reshape(HKV * HD, D)
    in_maps = []
    for c in range(NCORES):
        in_maps.append({
            "xT": xT,
            "wqT": bf(wq_p[EL * c:EL * (c + 1), :].T),
            "wkT": bf(wk_p[HD * c:HD * (c + 1), :].T),
            "wvT": bf(wv[HD * c:HD * (c + 1), :].T),
            "woT": bf(wo[EL * c:EL * (c + 1), :].T),
            "ckT": bf(cache_k[:, :, c, :].transpose(0, 2, 1)[:, PERM, :]),
            "cv": bf(cache_v[:, :, c, :]),
            "cosT": cosT, "sinT": sinT,
        })
    return in_maps


def assemble_out(results):
    return np.concatenate(
        [results[c]["out"].T for c in range(NCORES)], axis=1
    ).reshape(B, S, D)


def kernel(x, wq, wk, wv, wo, freqs_cos, freqs_sin, cache_k, cache_v,
           start_pos=START, **_ignored):
    assert x.shape == (B, S, D) and int(start_pos) == START
    nc = _get_nc(1)
    in_maps = make_in_maps(np.asarray(x, np.float32), np.asarray(wq, np.float32),
                           np.asarray(wk, np.float32), np.asarray(wv, np.float32),
                           np.asarray(wo, np.float32),
                           np.asarray(freqs_cos, np.float32),
                           np.asarray(freqs_sin, np.float32),
                           np.asarray(cache_k, np.float32),
                           np.asarray(cache_v, np.float32))
    res = run_bass_kernel_spmd(nc, in_maps, core_ids=list(range(NCORES)),
                               trace=False)
    return assemble_out(res.results)

